# revision 7
# baseline (speedup 1.0000x reference)
"""Longformer sliding-window attention on 8 trn2 NeuronCores.

B=2, H=12, L=4096, D=64, one-sided window w=256 (full window 513).
Shard: 24 (b,h) pairs -> 3 heads per core (data/head parallel; local
attention has no cross-(batch,head) interaction). Heads 0,1 share one
[128, L] q/k SBUF tile loaded with full-width DMA; head 1's matmul
operands read partitions 64:128.

Per-core algorithm (per head, 32 key-blocks of 128 keys):
  S^T[kb] = (K_blk)^T scores: (128 keys x <=640 queries) via bf16 matmuls
            (lhsT = K^T block (64,128), rhs = Q^T span (64,<=640))
  P^T = exp(S/8) on ScalarE (PSUM->SBUF, bf16), band edges masked
            multiplicatively on VectorE (two 128x128 triangle masks)
  ctx/denoms: PV matmul with V2 = [ones | V] (128 keys x 128):
            out rows 0:64 = softmax denominator replicated 64x,
            rows 64:128 = unnormalized ctx^T -- accumulated in (128,512)
            PSUM tiles per 512-query chunk.
  normalize: recip on VectorE, DMA-shift up to partitions 64:128,
            ctx^T * recip -> bf16 SBUF -> DMA.

REPEAT>1 wraps the per-head body in a device-side For_i loop (4 copies
per body so adjacent computations pipeline across the back-edge barrier)
so the wall-clock delta between two REPEAT values measures pure
on-device iteration time (NEFF size is R-independent).

Output assembled host-side: (24, 64, 4096) bf16 -> (2,12,4096,64) f32.
"""

import sys

sys.path.insert(0, "/opt/trn_rl_repo")

import numpy as np
import ml_dtypes

B, H, L, D = 2, 12, 4096, 64
W = 256            # one-sided window
NCORES = 8
HPC = (B * H) // NCORES   # heads per core = 3
BLK = 128                 # key block (partition dim)
NB = L // BLK             # 32 key blocks per head
SPAN = 2 * W + BLK        # 640 query columns per key block
CTXW = 512                # ctx psum tile width (1 bank)
NT = L // CTXW            # 8 ctx tiles per head

_CACHE = {}
REPEAT = 1           # device-side For_i trip count (timing harness lever)


def _build_program():
    import concourse.bacc as bacc
    import concourse.bass as bass
    import concourse.mybir as mybir
    import concourse.tile as tile

    f32 = mybir.dt.float32
    bf16 = mybir.dt.bfloat16

    nc = bacc.Bacc("TRN2", target_bir_lowering=False, debug=False)

    q01_d = nc.dram_tensor("q01", [2 * D, L], bf16, kind="ExternalInput").ap()
    k01_d = nc.dram_tensor("k01", [2 * D, L], bf16, kind="ExternalInput").ap()
    q2_d = nc.dram_tensor("q2", [D, L], bf16, kind="ExternalInput").ap()
    k2_d = nc.dram_tensor("k2", [D, L], bf16, kind="ExternalInput").ap()
    v_d = nc.dram_tensor("v", [HPC, BLK, NB, 2 * D], bf16, kind="ExternalInput").ap()
    em_d = nc.dram_tensor("masks", [BLK, 2, BLK], bf16, kind="ExternalInput").ap()
    out_d = nc.dram_tensor("out", [HPC, D, L], bf16, kind="ExternalOutput").ap()

    # per-(kb) geometry
    geo = []
    for kb in range(NB):
        K0 = BLK * kb
        qbase = K0 - W
        qlo = max(0, qbase)
        qhi = min(L, K0 + BLK + W)
        geo.append((K0, qbase, qlo, qhi))

    # ctx-tile contributors
    contrib = {t: [] for t in range(NT)}
    for kb, (K0, qbase, qlo, qhi) in enumerate(geo):
        for t in range(qlo // CTXW, (qhi - 1) // CTXW + 1):
            contrib[t].append(kb)
    last_kb = {t: kbs[-1] for t, kbs in contrib.items()}

    with tile.TileContext(nc) as tc:
        with (
            tc.tile_pool(name="const", bufs=1) as constp,
            tc.tile_pool(name="qk01", bufs=2) as qk01p,
            tc.tile_pool(name="qk2", bufs=2) as qk2p,
            tc.tile_pool(name="vp", bufs=3) as vp,
            tc.tile_pool(name="pt", bufs=4) as ptp,
            tc.tile_pool(name="outb", bufs=2) as outp,
            tc.tile_pool(name="rp", bufs=4) as rp,
            tc.tile_pool(name="st", bufs=2, space="PSUM") as stp,
            tc.tile_pool(name="ctx", bufs=4, space="PSUM") as ctxp,
        ):
            em = constp.tile([BLK, 2, BLK], bf16)
            nc.sync.dma_start(out=em, in_=em_d)

            def head_body(h):
                NCH = 4
                CH = L // NCH
                if h == 0:
                    qt_full = qk01p.tile([2 * D, L], bf16, name="q01", tag="q01")
                    kt_full = qk01p.tile([2 * D, L], bf16, name="k01", tag="k01")
                    for c in range(NCH):
                        sl = slice(c * CH, (c + 1) * CH)
                        nc.sync.dma_start(out=kt_full[:, sl], in_=k01_d[:, sl])
                        nc.sync.dma_start(out=qt_full[:, sl], in_=q01_d[:, sl])
                    head_body.qk01 = (qt_full, kt_full)
                    qtb_t = qt_full[0:D, :]
                    ktb_t = kt_full[0:D, :]
                elif h == 1:
                    qt_full, kt_full = head_body.qk01
                    qtb_t = qt_full[D : 2 * D, :]
                    ktb_t = kt_full[D : 2 * D, :]
                else:
                    qtb_t = qk2p.tile([D, L], bf16, name="q2", tag="q2")
                    ktb_t = qk2p.tile([D, L], bf16, name="k2", tag="k2")
                    for c in range(NCH):
                        sl = slice(c * CH, (c + 1) * CH)
                        nc.sync.dma_start(out=ktb_t[:, sl], in_=k2_d[:, sl])
                        nc.sync.dma_start(out=qtb_t[:, sl], in_=q2_d[:, sl])
                v2 = vp.tile([BLK, NB, 2 * D], bf16)
                for c in range(NCH):
                    bsl = slice(c * (NB // NCH), (c + 1) * (NB // NCH))
                    nc.sync.dma_start(out=v2[:, bsl, :], in_=v_d[h][:, bsl])

                outbuf = outp.tile([2 * D, L], bf16)
                ctx_tiles = {}
                ctx_started = set()

                def emit_qk_exp(kb):
                    K0, qbase, qlo, qhi = geo[kb]
                    c_lo, c_hi = qlo - qbase, qhi - qbase
                    st = stp.tile([BLK, SPAN], f32)
                    if c_lo < 512:
                        a, b = c_lo, min(512, c_hi)
                        nc.tensor.matmul(
                            st[:, a:b],
                            ktb_t[:, K0 : K0 + BLK],
                            qtb_t[:, qbase + a : qbase + b],
                            start=True,
                            stop=True,
                        )
                    if c_hi > 512:
                        nc.tensor.matmul(
                            st[:, 512:c_hi],
                            ktb_t[:, K0 : K0 + BLK],
                            qtb_t[:, qbase + 512 : qbase + c_hi],
                            start=True,
                            stop=True,
                        )
                    pt = ptp.tile([BLK, SPAN], bf16)
                    nc.scalar.activation(
                        pt[:, c_lo:c_hi],
                        st[:, c_lo:c_hi],
                        mybir.ActivationFunctionType.Exp,
                        scale=float(1.0 / np.sqrt(D)),
                    )
                    ptr = pt.rearrange("p (s c) -> p s c", c=BLK)
                    if c_lo == 0 and c_hi == SPAN:
                        # both triangle masks in one strided op (cols 0:128 + 512:640)
                        pte = bass.AP(
                            tensor=pt.tensor,
                            offset=pt.offset,
                            ap=[pt.ap[0], [4 * BLK, 2], [1, BLK]],
                        )
                        nc.vector.tensor_mul(pte, pte, em)
                    elif c_lo == 0:
                        nc.vector.tensor_mul(ptr[:, 0, :], ptr[:, 0, :], em[:, 0, :])
                    elif c_hi == SPAN:
                        nc.vector.tensor_mul(ptr[:, 4, :], ptr[:, 4, :], em[:, 1, :])
                    return pt

                fin_q = []
                mul_q = []

                def emit_pv(kb, pt):
                    K0, qbase, qlo, qhi = geo[kb]
                    for t in range(qlo // CTXW, (qhi - 1) // CTXW + 1):
                        a = max(qlo, CTXW * t)
                        b = min(qhi, CTXW * (t + 1))
                        if t not in ctx_tiles:
                            ctx_tiles[t] = ctxp.tile(
                                [BLK, CTXW], f32, name="ctx_t", tag="ctx_t"
                            )
                        first = t not in ctx_started
                        ctx_started.add(t)
                        nc.tensor.matmul(
                            ctx_tiles[t][:, a - CTXW * t : b - CTXW * t],
                            v2[:, kb, :],
                            pt[:, a - qbase : b - qbase],
                            start=first,
                            stop=(kb == last_kb[t]),
                        )
                    for t in list(ctx_tiles):
                        if last_kb[t] == kb:
                            fin_q.append((t, ctx_tiles.pop(t)))

                def flush_fin():
                    # two-stage deferral so neither the recip's wait on the
                    # PE nor the mul's wait on the shift DMA head-of-line
                    # blocks the next mask in the in-order DVE queue:
                    # stage 2 first (muls whose shift DMA was issued one
                    # pipeline step ago), then stage 1 for newly completed
                    # tiles (recip + shift-DMA issue).
                    for t, ct, rhi in mul_q:
                        ob = outbuf[D : 2 * D, CTXW * t : CTXW * (t + 1)]
                        nc.vector.tensor_mul(
                            ob, ct[D : 2 * D, :], rhi[D : 2 * D, :]
                        )
                    mul_q.clear()
                    for t, ct in fin_q:
                        # denoms on partitions 0:64 (custom-DVE recip
                        # needs base partition 0); DMA-shift recip up
                        # to 64:128 where the ctx rows live, mul there
                        rlo = rp.tile([D, CTXW], f32, name="rlo")
                        nc.vector.reciprocal_approx_fast(
                            out=rlo, in_=ct[0:D, :]
                        )
                        rhi = rp.tile([2 * D, CTXW], f32, name="rhi")
                        nc.sync.dma_start(out=rhi[D : 2 * D, :], in_=rlo)
                        mul_q.append((t, ct, rhi))
                    fin_q.clear()

                # software pipeline: PV for block kb is emitted after the
                # QK+exp for block kb+1, so the in-order PE stream never
                # head-of-line blocks on the exp/mask chain.
                pending = None
                for kb in range(NB):
                    pt = emit_qk_exp(kb)
                    flush_fin()
                    if pending is not None:
                        emit_pv(pending[0], pending[1])
                    pending = (kb, pt)
                emit_pv(pending[0], pending[1])
                flush_fin()
                flush_fin()

                nc.sync.dma_start(out=out_d[h], in_=outbuf[D : 2 * D, :])

            # REPEAT computations total. Two copies per For_i body so
            # adjacent computations pipeline across the body boundary
            # (the back-edge is a full barrier); REPEAT > 1 must be even
            # to use the unrolled form.
            if REPEAT == 1:
                for h in range(HPC):
                    head_body(h)
            elif REPEAT % 4 == 0:
                with tc.For_i(0, REPEAT // 4, 1):
                    for _ in range(4):
                        for h in range(HPC):
                            head_body(h)
            elif REPEAT % 2 == 0:
                with tc.For_i(0, REPEAT // 2, 1):
                    for h in range(HPC):
                        head_body(h)
                    for h in range(HPC):
                        head_body(h)
            else:
                with tc.For_i(0, REPEAT, 1):
                    for h in range(HPC):
                        head_body(h)

    nc.compile()
    return nc


def _get_nc():
    if "nc" not in _CACHE:
        _CACHE["nc"] = _build_program()
    return _CACHE["nc"]


def _host_prep(q, k, v):
    qf = np.ascontiguousarray(
        np.asarray(q, dtype=np.float32).transpose(0, 1, 3, 2)
    ).reshape(B * H, D, L).astype(ml_dtypes.bfloat16)
    kf = np.ascontiguousarray(
        np.asarray(k, dtype=np.float32).transpose(0, 1, 3, 2)
    ).reshape(B * H, D, L).astype(ml_dtypes.bfloat16)

    vperm = (
        np.asarray(v, dtype=np.float32)
        .reshape(B * H, NB, BLK, D)
        .transpose(0, 2, 1, 3)
        .astype(ml_dtypes.bfloat16)
    )
    v2 = np.empty((B * H, BLK, NB, 2 * D), dtype=ml_dtypes.bfloat16)
    v2[..., :D] = ml_dtypes.bfloat16(1.0)
    v2[..., D:] = vperm

    i = np.arange(BLK)
    em = np.zeros((BLK, 2, BLK), dtype=ml_dtypes.bfloat16)
    em[:, 0, :] = (i[None, :] >= i[:, None]).astype(ml_dtypes.bfloat16)  # left: col>=row
    em[:, 1, :] = (i[None, :] <= i[:, None]).astype(ml_dtypes.bfloat16)  # right: col<=row

    in_maps = []
    for c in range(NCORES):
        s0 = c * HPC
        in_maps.append(
            {
                "q01": np.ascontiguousarray(qf[s0 : s0 + 2].reshape(2 * D, L)),
                "k01": np.ascontiguousarray(kf[s0 : s0 + 2].reshape(2 * D, L)),
                "q2": np.ascontiguousarray(qf[s0 + 2]),
                "k2": np.ascontiguousarray(kf[s0 + 2]),
                "v": np.ascontiguousarray(v2[s0 : s0 + HPC]),
                "masks": em,
            }
        )
    return in_maps


def kernel(q, k, v, padding_mask):
    from concourse.bass_utils import run_bass_kernel_spmd

    pm = np.asarray(padding_mask)
    assert pm.all(), "kernel specialized for all-ones padding mask"

    nc = _get_nc()
    in_maps = _host_prep(q, k, v)
    res = run_bass_kernel_spmd(nc, in_maps, core_ids=list(range(NCORES)))
    outs = [res.results[c]["out"] for c in range(NCORES)]  # each (HPC, 64, 4096) bf16
    full = np.concatenate(outs, axis=0)                     # (24, 64, 4096)
    ctx = full.astype(np.float32).transpose(0, 2, 1).reshape(B, H, L, D)
    return np.ascontiguousarray(ctx)
